# revision 1
# baseline (speedup 1.0000x reference)
"""Trainium2 Bass kernel for grouped per-atom MLPs (AtomicNN energy eval).

Math: e[s, a] = W3[a].T tanh(W2[a].T tanh(W1[a].T g[s,a] + b1[a]) + b2[a]) + b3[a]
Shapes: g [4096, 1024, 5], per-atom MLP 5 -> 64 -> 64 -> 1.

Strategy (8 NeuronCores, SPMD):
 - Shard the atom axis: core c owns atoms [128c, 128c+128). All 4096 structs
   stream through each core (expert-style parallelism; weights are small and
   unique per atom, so this avoids replicating the weight DMA 8x).
 - Atoms are processed in pairs (2x64 = 128 PE rows/cols). The struct axis is
   the matmul moving (N) dimension, 512 columns per matmul (one PSUM bank).
 - Layer 1: lhsT = blockdiag(W1[2p], W1[2p+1]) with an extra bias row
   ([11, 128]); rhs = transposed g pair tile + ones row ([11, 512]) -> fuses
   the b1 add into the matmul.
 - Layer 2: lhsT = blockdiag(W2[2p], W2[2p+1]) ([128, 128]).
 - Layer 3: 64 accumulating matmuls (one per pair) into a single PSUM bank:
   lhsT[:, 2p] = [W3[2p]; 0], lhsT[:, 2p+1] = [0; W3[2p+1]] -> builds the
   [128 atoms, 512 structs] transposed-output block directly.
 - tanh on the Scalar (ACT) engine, batched over multiple PSUM banks per
   instruction (4 pairs for layer 1, 2 pairs for layer 2) to amortize the
   per-instruction overhead; ACT is the bottleneck engine for this problem.
 - Matmul dtype float32r (~1.5e-4 rel err vs fp32; same net speed as bf16
   here since the PE runs throttled at 1.2 GHz under this dependency mix).
 - g is pre-transposed host-side to [chunk, 11, pair, 512] so all device DMAs
   are contiguous-2KB-row strided loads.
"""

from contextlib import ExitStack

import numpy as np

S, A, D, H = 4096, 1024, 5, 64
NCORES = 8
ACORE = A // NCORES  # 128 atoms per core
NPAIR = ACORE // 2  # 64 atom pairs per core
NS = 512  # struct chunk = one PSUM bank of fp32
NCHUNK = S // NS  # 8
KG = D * 2 + 1  # 11: two atoms' descriptors + ones row for the b1 fold
G1 = 2  # pairs per layer-1 tanh batch (2 PSUM banks, double-buffered)
G2 = 2  # pairs per layer-2 tanh batch (2 PSUM banks)

_compiled = {}

MM_DT = "float32r"  # matmul operand dtype: bfloat16 | float32r


def _build(with_b2):
    import concourse.tile as tile
    import concourse.mybir as mybir
    from concourse import bacc

    dt = mybir.dt
    mdt = getattr(dt, MM_DT)
    Tanh = mybir.ActivationFunctionType.Tanh

    nc = bacc.Bacc(
        "TRN2", target_bir_lowering=False, debug=False, num_devices=NCORES
    )
    gt = nc.declare_dram_parameter(
        "gt", [NCHUNK, KG, NPAIR, NS], mdt, isOutput=False
    )
    w1 = nc.declare_dram_parameter(
        "w1", [KG, NPAIR * 128], mdt, isOutput=False
    )
    w2 = nc.declare_dram_parameter(
        "w2", [128, NPAIR * 128], mdt, isOutput=False
    )
    w3 = nc.declare_dram_parameter(
        "w3", [128, NPAIR * 128], mdt, isOutput=False
    )
    if with_b2:
        b2d = nc.declare_dram_parameter("b2d", [128, NPAIR], dt.float32, isOutput=False)
    b3d = nc.declare_dram_parameter("b3d", [128, 1], dt.float32, isOutput=False)
    eo = nc.declare_dram_parameter("eo", [128, S], dt.float32, isOutput=True)

    with tile.TileContext(nc) as tc, ExitStack() as ctx:
        wp = ctx.enter_context(tc.tile_pool(name="wp", bufs=1))
        gp = ctx.enter_context(tc.tile_pool(name="gp", bufs=2))
        h1p = ctx.enter_context(tc.tile_pool(name="h1p", bufs=3))
        h2p = ctx.enter_context(tc.tile_pool(name="h2p", bufs=3))
        eop = ctx.enter_context(tc.tile_pool(name="eop", bufs=2))
        z1p = ctx.enter_context(tc.tile_pool(name="z1p", bufs=2, space="PSUM"))
        z2p = ctx.enter_context(tc.tile_pool(name="z2p", bufs=1, space="PSUM"))
        etp = ctx.enter_context(tc.tile_pool(name="etp", bufs=2, space="PSUM"))

        w1t = wp.tile([KG, NPAIR * 128], mdt)
        nc.sync.dma_start(w1t[:], w1[:])
        w2t = wp.tile([128, NPAIR * 128], mdt)
        nc.gpsimd.dma_start(w2t[:], w2[:])
        w3t = wp.tile([128, NPAIR * 128], mdt)
        nc.gpsimd.dma_start(w3t[:], w3[:])
        b3t = wp.tile([128, 1], dt.float32)
        nc.sync.dma_start(b3t[:], b3d[:])
        if with_b2:
            b2t = wp.tile([128, NPAIR], dt.float32)
            nc.sync.dma_start(b2t[:], b2d[:])

        NGRP = NPAIR // G1  # groups per chunk
        et_tiles = {}

        HALF = NPAIR // 4  # pairs per staged g DMA (quarter chunk)
        NHALF = NCHUNK * 4
        gstage = {}

        def ensure_half(hq):
            if hq in gstage or hq >= NHALF:
                return
            hc, hi = divmod(hq, 4)
            gs = gp.tile([KG, HALF * NS], mdt, name=f"gs{hq}", tag="gs")
            p0 = hi * HALF
            nc.sync.dma_start(gs[:], gt[hc, :, p0 : p0 + HALF, :])
            gstage[hq] = gs

        def stage_front(c, g):
            """Layer-1 matmuls + tanh1 from the staged g (prefetch one half
            ahead; the gs pool double-buffers)."""
            hq = c * 4 + (g * G1) // HALF
            ensure_half(hq)
            ensure_half(hq + 1)
            gs = gstage[hq]
            half = (c, (g * G1) // HALF)
            z1 = z1p.tile([128, G1 * NS], dt.float32)
            for i in range(G1):
                p = g * G1 + i
                off = (p - half[1] * HALF) * NS
                nc.tensor.matmul(
                    z1[:, i * NS : (i + 1) * NS],
                    w1t[:, p * 128 : (p + 1) * 128],
                    gs[:, off : off + NS],
                    start=True,
                    stop=True,
                )
            h1 = h1p.tile([128, G1 * NS], mdt)
            nc.scalar.activation(h1[:], z1[:], Tanh)
            return h1

        def stage_back(c, g, h1):
            """Layer-2 matmuls (batched z2), tanh2, layer-3 accumulation;
            flush et at chunk end."""
            if c not in et_tiles:
                et_tiles[c] = etp.tile([128, NS], dt.float32, name=f"et{c}", tag="et")
            et = et_tiles[c]
            for j in range(G1 // G2):
                z2 = z2p.tile([128, G2 * NS], dt.float32, name=f"z2_{c}_{g}_{j}", tag="z2")
                for k in range(G2):
                    p = g * G1 + j * G2 + k
                    q = j * G2 + k
                    nc.tensor.matmul(
                        z2[:, k * NS : (k + 1) * NS],
                        w2t[:, p * 128 : (p + 1) * 128],
                        h1[:, q * NS : (q + 1) * NS],
                        start=True,
                        stop=True,
                    )
                if with_b2:
                    for k in range(G2):
                        p = g * G1 + j * G2 + k
                        nc.vector.tensor_scalar_add(
                            z2[:, k * NS : (k + 1) * NS],
                            z2[:, k * NS : (k + 1) * NS],
                            b2t[:, p : p + 1],
                        )
                h2 = h2p.tile([128, G2 * NS], mdt, name=f"h2_{c}_{g}_{j}", tag="h2")
                nc.scalar.activation(h2[:], z2[:], Tanh)
                for k in range(G2):
                    p = g * G1 + j * G2 + k
                    nc.tensor.matmul(
                        et[:],
                        w3t[:, p * 128 : (p + 1) * 128],
                        h2[:, k * NS : (k + 1) * NS],
                        start=(p == 0),
                        stop=(p == NPAIR - 1),
                    )
            if g == NGRP - 1:
                eot = eop.tile([128, NS], dt.float32)
                nc.vector.tensor_scalar_add(eot[:], et[:], b3t[:])
                nc.sync.dma_start(eo[:, c * NS : (c + 1) * NS], eot[:])
                del et_tiles[c]

        # Deferred PE warm-up: a dense matmul burst anchored on the w2t
        # weight DMA, so it runs right before the first real matmuls with no
        # idle gap in between -- the HAM un-throttles to 2.4 GHz during the
        # burst and the sub-us gaps of the main stream never re-throttle it.
        zwu = z1p.tile([128, NS], dt.float32, name="zwu", tag="z1")
        for _ in range(24):
            nc.tensor.matmul(
                zwu[:], w2t[:, 0:128], w2t[:, 1024:1536], start=True, stop=True
            )

        # Software pipeline: issue group q's front stage before group q-1's
        # back stage so the ACT engine always has an independent tanh queued.
        pending = None
        for q in range(NCHUNK * NGRP):
            c, g = divmod(q, NGRP)
            h1 = stage_front(c, g)
            if pending is not None:
                stage_back(*pending)
            pending = (c, g, h1)
        stage_back(*pending)
    nc.compile()
    return nc


def _prep_core(c, g, W1, b1, W2, b2, W3, b3, with_b2):
    import ml_dtypes

    at = slice(c * ACORE, (c + 1) * ACORE)
    f32 = np.float32
    mdt = ml_dtypes.bfloat16 if MM_DT == "bfloat16" else np.float32

    # gt: [NCHUNK, 11, NPAIR, NS]; row r<10: descriptor d=r%5 of even/odd atom
    # of each pair; row 10: ones (streams the b1 fold).
    gc = g[:, at, :]  # [S, 128, 5]
    gT = np.ascontiguousarray(gc.transpose(1, 2, 0))  # [128, 5, S]
    gT = gT.reshape(NPAIR, 2 * D, S)  # [64, 10, S]
    gt = np.empty((NCHUNK, KG, NPAIR, NS), dtype=f32)
    # [64, 10, S] -> [10, 64, NCHUNK, NS] -> chunk-major
    gt[:, : 2 * D] = gT.transpose(1, 0, 2).reshape(2 * D, NPAIR, NCHUNK, NS).transpose(2, 0, 1, 3)
    gt[:, 2 * D] = 1.0

    W1c, b1c = W1[at], b1[at]  # [128, 5, 64], [128, 64]
    w1a = np.zeros((NPAIR, KG, 128), dtype=f32)
    w1a[:, :D, :H] = W1c[0::2]
    w1a[:, D : 2 * D, H:] = W1c[1::2]
    w1a[:, 2 * D, :H] = b1c[0::2]
    w1a[:, 2 * D, H:] = b1c[1::2]
    w1d = np.ascontiguousarray(w1a.transpose(1, 0, 2)).reshape(KG, NPAIR * 128)

    W2c = W2[at]  # [128, 64, 64]
    w2a = np.zeros((NPAIR, 128, 128), dtype=f32)
    w2a[:, :H, :H] = W2c[0::2]
    w2a[:, H:, H:] = W2c[1::2]
    w2d = np.ascontiguousarray(w2a.transpose(1, 0, 2)).reshape(128, NPAIR * 128)

    W3c = W3[at][..., 0]  # [128, 64]
    w3a = np.zeros((NPAIR, 128, 128), dtype=f32)
    for p in range(NPAIR):
        w3a[p, :H, 2 * p] = W3c[2 * p]
        w3a[p, H:, 2 * p + 1] = W3c[2 * p + 1]
    w3d = np.ascontiguousarray(w3a.transpose(1, 0, 2)).reshape(128, NPAIR * 128)

    in_map = {
        "gt": np.ascontiguousarray(gt).astype(mdt),
        "w1": w1d.astype(mdt),
        "w2": w2d.astype(mdt),
        "w3": w3d.astype(mdt),
        "b3d": np.ascontiguousarray(b3[at]).astype(f32),
    }
    if with_b2:
        b2c = b2[at]  # [128, 64]
        in_map["b2d"] = np.ascontiguousarray(
            np.concatenate([b2c[0::2].T, b2c[1::2].T], axis=0)
        ).astype(f32)
    return in_map


def kernel(g, W1, b1, W2, b2, W3, b3):
    from concourse.bass_utils import run_bass_kernel_spmd

    g = np.asarray(g, dtype=np.float32)
    W1 = np.asarray(W1, dtype=np.float32)
    b1 = np.asarray(b1, dtype=np.float32)
    W2 = np.asarray(W2, dtype=np.float32)
    b2 = np.asarray(b2, dtype=np.float32)
    W3 = np.asarray(W3, dtype=np.float32)
    b3 = np.asarray(b3, dtype=np.float32)

    with_b2 = bool(np.any(b2))
    if with_b2 not in _compiled:
        _compiled[with_b2] = _build(with_b2)
    nc = _compiled[with_b2]

    in_maps = [
        _prep_core(c, g, W1, b1, W2, b2, W3, b3, with_b2) for c in range(NCORES)
    ]
    res = run_bass_kernel_spmd(nc, in_maps, list(range(NCORES)))

    e = np.empty((S, A), dtype=np.float32)
    for c in range(NCORES):
        e[:, c * ACORE : (c + 1) * ACORE] = res.results[c]["eo"].T
    return e



# revision 10
# speedup vs baseline: 1.0831x; 1.0831x over previous
"""Trainium2 Bass kernel for grouped per-atom MLPs (AtomicNN energy eval).

Math: e[s, a] = W3[a].T tanh(W2[a].T tanh(W1[a].T g[s,a] + b1[a]) + b2[a]) + b3[a]
Shapes: g [4096, 1024, 5], per-atom MLP 5 -> 64 -> 64 -> 1.

Strategy (8 NeuronCores, SPMD, atom-sharded: core c owns atoms [128c, 128c+128)):
 - All matmuls in bf16 (fp32r streams at 1.5 cyc/row on TRN2; bf16 = 1.0 and
   enables the compiler's fast-weight-load path).
 - Waves of 2 atom-pairs over 512-struct chunks. Layer-1 matmuls are
   row-tiled (K=11 per pair at partition offsets 32*rg) so two/four K=11
   matmuls run concurrently in the PE array.
 - Layer 3 (e = W3.T h2) runs as a chunk-end burst of column-tiled matmuls:
   4 concurrent M=32 matmuls (one per 32-atom column group), 16 accumulation
   steps, writing the [128 atoms, 512 structs] block in one PSUM bank. The
   bank borrows a z1 pool slot for ~4us at each chunk boundary, keeping total
   PSUM usage at 8 banks (z1 ring 4 + z2 ring 4).
 - tanh1 on the ACT engine (exact, table-based). tanh2 mostly on the Vector
   engine via a custom fused DVE op: clip(x*((a*u + b)*u + c), -s, s) with
   u = x^2 -- an 8-stage minimax fit good to ~6e-3 RMS over the z2
   distribution; every few waves tanh2 goes to ACT instead to balance load.
 - A ~4.5us matmul warm-up burst anchored on the first g-chunk DMA flips the
   PE HAM clock gate to 8/8 (2.4 GHz) right before the pipeline starts and
   the steady-state stream keeps it warm.
"""

from contextlib import ExitStack

import numpy as np

S, A, D, H = 4096, 1024, 5, 64
NCORES = 8
ACORE = A // NCORES  # 128 atoms per core
NPAIR = ACORE // 2  # 64 atom pairs per core
NS = 512  # struct chunk = one PSUM bank of fp32
NCHUNK = S // NS  # 8
KG = D * 2 + 1  # 11 rows per pair: two atoms' descriptors + ones row (b1 fold)
NWAVE = 32  # waves per chunk, 2 pairs each
NT = 16  # pairs per l3 column group

# Poly-tanh coefficients (minimax, weighted by z2 ~ N(0, 6.8)):
# tanh(x) ~= clip(x*((PA*u + PB)*u + PC), -PS, PS), u = x*x
PA = 0.020941360690687257
PB = -0.2006771642928505
PC = 0.9497521924685897
PS = 0.9979386257948004

ACT_TANH2_EVERY = 11  # every k-th wave's tanh2 runs on ACT instead of DVE

_compiled = {}
_dve_registered = []


def _register_tanh_op():
    """Register the fused poly-tanh custom DVE op (runtime extension of the
    documented dve_ops registry; sha self-pinned)."""
    if _dve_registered:
        return _dve_registered[0]
    import concourse.dve_ops as dve_ops
    from concourse.dve_ops import DveOp
    from concourse.dve_spec import Spec, Src0, Src1, C0, C1, C2, Zero, lower, maxx, minn, sq
    from concourse.dve_uop import DveOpSpec

    name = "TANH_POLY_ANT"
    u = sq(Src0)
    spec = Spec(
        body=minn(maxx(Src0 * ((u * C0 + C1) * u + Src1), Zero - C2), C2),
        reference=lambda in0, in1, s0, s1, imm2: np.clip(
            in0.astype(np.float32)
            * (
                (np.square(in0.astype(np.float32)) * s0 + s1)
                * np.square(in0.astype(np.float32))
                + in1
            ),
            -imm2,
            imm2,
        ),
    )
    if name not in dve_ops._SUB_OPCODE_FOR_NAME:
        row = max(dve_ops._SUB_OPCODE_FOR_NAME.values()) + 1
        assert row < 0x20
        dve_ops._SUB_OPCODE_FOR_NAME[name] = row
    shas = {}
    for ver in ("v3", "v4"):
        s = DveOpSpec(
            name=name,
            opcode=dve_ops._SUB_OPCODE_FOR_NAME[name],
            uops=lower(spec, ver=ver),
            rd1_en=True,
        )
        shas[ver] = s.sha(ver)
    op = DveOp(name, spec, subdim=False, uops_sha=shas)
    if not any(o.name == name for o in dve_ops.OPS):
        dve_ops.OPS.append(op)
    dve_ops.CUSTOM_DVE_SPECS[name] = spec
    _dve_registered.append(op)
    return op


def _build(with_b2):
    import concourse.tile as tile
    import concourse.mybir as mybir
    from concourse import bacc

    tanh_op = _register_tanh_op()

    dt = mybir.dt
    mdt = dt.bfloat16
    Tanh = mybir.ActivationFunctionType.Tanh

    nc = bacc.Bacc(
        "TRN2", target_bir_lowering=False, debug=False, num_devices=NCORES
    )
    # gt: [chunk, rowgroup, 11, 16 wave-slots, NS] (see _prep_core)
    gt = nc.declare_dram_parameter("gt", [NCHUNK, 4, KG, NT, NS], mdt, isOutput=False)
    w1 = nc.declare_dram_parameter("w1", [128, NT * 128], mdt, isOutput=False)
    w2 = nc.declare_dram_parameter("w2", [128, NPAIR * 128], mdt, isOutput=False)
    w3 = nc.declare_dram_parameter("w3", [128, NPAIR * 32], mdt, isOutput=False)
    b3d = nc.declare_dram_parameter("b3d", [128, 1], dt.float32, isOutput=False)
    if with_b2:
        b2d = nc.declare_dram_parameter("b2d", [128, NPAIR], dt.float32, isOutput=False)
    eo = nc.declare_dram_parameter("eo", [128, S], dt.float32, isOutput=True)

    with tile.TileContext(nc) as tc, ExitStack() as ctx:
        wp = ctx.enter_context(tc.tile_pool(name="wp", bufs=1))
        gp = ctx.enter_context(tc.tile_pool(name="gp", bufs=2))
        h1p = ctx.enter_context(tc.tile_pool(name="h1p", bufs=3))
        h2p = ctx.enter_context(tc.tile_pool(name="h2p", bufs=36))
        eop = ctx.enter_context(tc.tile_pool(name="eop", bufs=2))
        z1p = ctx.enter_context(tc.tile_pool(name="z1p", bufs=2, space="PSUM"))
        z2p = ctx.enter_context(tc.tile_pool(name="z2p", bufs=2, space="PSUM"))

        w1t = wp.tile([128, NT * 128], mdt)
        nc.sync.dma_start(w1t[:], w1[:])
        w2t = wp.tile([128, NPAIR * 128], mdt)
        nc.gpsimd.dma_start(w2t[:], w2[:])
        w3t = wp.tile([128, NPAIR * 32], mdt)
        nc.gpsimd.dma_start(w3t[:], w3[:])
        b3t = wp.tile([128, 1], dt.float32)
        nc.sync.dma_start(b3t[:], b3d[:])
        if with_b2:
            b2t = wp.tile([128, NPAIR], dt.float32)
            nc.sync.dma_start(b2t[:], b2d[:])
        # per-partition broadcast of the poly "c" coefficient for the DVE op
        ct = wp.tile([128, 1], dt.float32)
        nc.gpsimd.memset(ct[:], PC)

        gstage = {}

        def ensure_chunk(c):
            if c in gstage or c >= NCHUNK:
                return
            gs = gp.tile([128, NT * NS], mdt, name=f"gs{c}", tag="gs")
            for rg in range(4):
                nc.sync.dma_start(gs[32 * rg : 32 * rg + KG, :], gt[c, rg])
            gstage[c] = gs

        ensure_chunk(0)

        # Warm-up: ~10 back-to-back N=512 matmuls anchored on the chunk-0 g
        # DMA -- ~4.5us of dense PE work flips the HAM clock gate to 8/8
        # right before the first real wave; steady-state gaps stay under the
        # ~3.4us re-throttle window so the PE stays at 2.4 GHz.
        zwu = z1p.tile([128, 1024], dt.float32, name="zwu", tag="z1")
        for _ in range(10):
            nc.tensor.matmul(
                zwu[:, 0:NS],
                w1t[0:KG, 0:128],
                gstage[0][0:KG, 0:NS],
                start=True,
                stop=True,
            )

        h2_tiles = {}  # pair index -> (tile, col slice start)

        def stage_l1(c, w):
            """Layer-1 wave: 2 row-tiled K=11 matmuls (pairs at row groups
            (0,1) for w<16, (2,3) for w>=16) into one 2-bank z1 tile."""
            ensure_chunk(c + 1)
            gs = gstage[c]
            t = w % NT
            z1 = z1p.tile([128, 1024], dt.float32, name=f"z1_{c}_{w}", tag="z1")
            for s in range(2):
                rg = 2 * (w // NT) + s
                nc.tensor.matmul(
                    z1[:, s * NS : (s + 1) * NS],
                    w1t[32 * rg : 32 * rg + KG, t * 128 : (t + 1) * 128],
                    gs[32 * rg : 32 * rg + KG, t * NS : (t + 1) * NS],
                    start=True,
                    stop=True,
                )
            h1 = h1p.tile([128, 1024], mdt, name=f"h1_{c}_{w}", tag="h1")
            nc.scalar.activation(h1[:], z1[:], Tanh)
            return h1

        def stage_l2(c, w, h1, q):
            """Layer-2 wave: 2 matmuls + tanh2 (custom DVE poly op, or exact
            ACT tanh every ACT_TANH2_EVERY waves / when b2 is folded)."""
            t = w % NT
            z2 = z2p.tile([128, 1024], dt.float32, name=f"z2_{c}_{w}", tag="z2")
            for s in range(2):
                rg = 2 * (w // NT) + s
                p = NT * rg + t
                nc.tensor.matmul(
                    z2[:, s * NS : (s + 1) * NS],
                    w2t[:, p * 128 : (p + 1) * 128],
                    h1[:, s * NS : (s + 1) * NS],
                    start=True,
                    stop=True,
                )
            if with_b2:
                for s in range(2):
                    rg = 2 * (w // NT) + s
                    p = NT * rg + t
                    nc.vector.tensor_scalar_add(
                        z2[:, s * NS : (s + 1) * NS],
                        z2[:, s * NS : (s + 1) * NS],
                        b2t[:, p : p + 1],
                    )
            h2 = h2p.tile([128, 1024], mdt, name=f"h2_{c}_{w}", tag="h2")
            if q % ACT_TANH2_EVERY == ACT_TANH2_EVERY - 1:
                nc.scalar.activation(h2[:], z2[:], Tanh)
            else:
                nc.vector._custom_dve(
                    tanh_op, out=h2[:], in0=z2[:], in1=ct[:], s0=PA, s1=PB, imm2=PS
                )
            for s in range(2):
                rg = 2 * (w // NT) + s
                h2_tiles[NT * rg + t] = (h2, s * NS)
            return h2

        def stage_l3(c):
            """Chunk-end burst: per accumulation step t, four column-tiled
            matmuls (one per 32-atom column group) run concurrently; the
            [128, 512] energy block accumulates in a z1-pool bank."""
            et = z1p.tile([128, 1024], dt.float32, name=f"et{c}", tag="z1")
            for t in range(NT):
                for gcol in range(4):
                    p = NT * gcol + t
                    h2, off = h2_tiles[p]
                    nc.tensor.matmul(
                        et[32 * gcol : 32 * gcol + 32, 0:NS],
                        w3t[:, p * 32 : (p + 1) * 32],
                        h2[:, off : off + NS],
                        start=(t == 0),
                        stop=(t == NT - 1),
                    )
            h2_tiles.clear()
            eot = eop.tile([128, NS], dt.float32)
            nc.vector.tensor_scalar_add(eot[:], et[:, 0:NS], b3t[:])
            nc.sync.dma_start(eo[:, c * NS : (c + 1) * NS], eot[:])

        # Software pipeline: wave q's layer-1 is issued before wave q-1's
        # layer-2 so every engine always has independent work queued.
        pending = None
        for q in range(NCHUNK * NWAVE):
            c, w = divmod(q, NWAVE)
            h1 = stage_l1(c, w)
            if pending is not None:
                pc, pw, ph1 = pending
                stage_l2(pc, pw, ph1, q - 1)
                if pw == NWAVE - 1:
                    stage_l3(pc)
            pending = (c, w, h1)
        pc, pw, ph1 = pending
        stage_l2(pc, pw, ph1, NCHUNK * NWAVE - 1)
        stage_l3(pc)
    nc.compile()
    return nc


def _prep_core(c, g, W1, b1, W2, b2, W3, b3, with_b2):
    import ml_dtypes

    at = slice(c * ACORE, (c + 1) * ACORE)
    f32 = np.float32
    bf16 = ml_dtypes.bfloat16

    # Pair P = atoms (2P, 2P+1); l3 column group gcol = P // 16, slot t = P % 16.
    # Wave w covers pairs (16*(2*(w//16)) + w%16, 16*(2*(w//16)+1) + w%16).
    gc = g[:, at, :]  # [S, 128, 5]
    # gt[c, rg, r, t, n]: descriptor row r of pair P = 16*rg + t at struct 512c+n.
    gT = np.ascontiguousarray(gc.transpose(1, 2, 0))  # [128, 5, S]
    gT = gT.reshape(NPAIR, 2, D, S)  # [pair, parity, d, S]
    gt = np.empty((NCHUNK, 4, KG, NT, NS), dtype=f32)
    # rows 0-4: even atom's descriptors; 5-9: odd atom's; 10: ones
    src = gT.reshape(4, NT, 2, D, NCHUNK, NS)  # [rg, t, parity, d, c, n]
    gt[:, :, 0:D] = src[:, :, 0].transpose(3, 0, 2, 1, 4)
    gt[:, :, D : 2 * D] = src[:, :, 1].transpose(3, 0, 2, 1, 4)
    gt[:, :, 2 * D] = 1.0

    W1c, b1c = W1[at], b1[at]  # [128, 5, 64], [128, 64]
    # w1t rows 32*rg + r, block t: pair P = 16*rg + t
    w1a = np.zeros((4, 32, NT, 128), dtype=f32)
    W1p = W1c.reshape(NPAIR, 2, D, H).reshape(4, NT, 2, D, H)
    b1p = b1c.reshape(NPAIR, 2, H).reshape(4, NT, 2, H)
    w1a[:, 0:D, :, 0:H] = W1p[:, :, 0].transpose(0, 2, 1, 3)
    w1a[:, D : 2 * D, :, H:] = W1p[:, :, 1].transpose(0, 2, 1, 3)
    w1a[:, 2 * D, :, 0:H] = b1p[:, :, 0]
    w1a[:, 2 * D, :, H:] = b1p[:, :, 1]
    w1d = w1a.reshape(128, NT * 128)

    W2c = W2[at]  # [128, 64, 64]
    w2a = np.zeros((NPAIR, 128, 128), dtype=f32)
    w2a[:, :H, :H] = W2c[0::2]
    w2a[:, H:, H:] = W2c[1::2]
    w2d = np.ascontiguousarray(w2a.transpose(1, 0, 2)).reshape(128, NPAIR * 128)

    W3c = W3[at][..., 0]  # [128, 64]
    w3a = np.zeros((NPAIR, 128, 32), dtype=f32)
    for p in range(NPAIR):
        t = p % NT
        w3a[p, :H, 2 * t] = W3c[2 * p]
        w3a[p, H:, 2 * t + 1] = W3c[2 * p + 1]
    w3d = np.ascontiguousarray(w3a.transpose(1, 0, 2)).reshape(128, NPAIR * 32)

    in_map = {
        "gt": np.ascontiguousarray(gt).astype(bf16),
        "w1": w1d.astype(bf16),
        "w2": w2d.astype(bf16),
        "w3": w3d.astype(bf16),
        "b3d": np.ascontiguousarray(b3[at]).astype(f32),
    }
    if with_b2:
        b2c = b2[at]  # [128, 64]
        in_map["b2d"] = np.ascontiguousarray(
            np.concatenate([b2c[0::2].T, b2c[1::2].T], axis=0)
        ).astype(f32)
    return in_map


def kernel(g, W1, b1, W2, b2, W3, b3):
    from concourse.bass_utils import run_bass_kernel_spmd

    g = np.asarray(g, dtype=np.float32)
    W1 = np.asarray(W1, dtype=np.float32)
    b1 = np.asarray(b1, dtype=np.float32)
    W2 = np.asarray(W2, dtype=np.float32)
    b2 = np.asarray(b2, dtype=np.float32)
    W3 = np.asarray(W3, dtype=np.float32)
    b3 = np.asarray(b3, dtype=np.float32)

    with_b2 = bool(np.any(b2))
    if with_b2 not in _compiled:
        _compiled[with_b2] = _build(with_b2)
    nc = _compiled[with_b2]

    in_maps = [
        _prep_core(c, g, W1, b1, W2, b2, W3, b3, with_b2) for c in range(NCORES)
    ]
    res = run_bass_kernel_spmd(nc, in_maps, list(range(NCORES)))

    e = np.empty((S, A), dtype=np.float32)
    for c in range(NCORES):
        e[:, c * ACORE : (c + 1) * ACORE] = res.results[c]["eo"].T
    return e


# revision 26
# speedup vs baseline: 2.0591x; 1.9011x over previous
"""Trainium2 Bass kernel for grouped per-atom MLPs (AtomicNN energy eval).

Math: e[s, a] = W3[a].T tanh(W2[a].T tanh(W1[a].T g[s,a] + b1[a]) + b2[a]) + b3[a]
Shapes: g [4096, 1024, 5], per-atom MLP 5 -> 64 -> 64 -> 1.

Strategy (8 NeuronCores, SPMD, atom-sharded: core c owns atoms [128c, 128c+128)):
 - All matmuls in bf16 (fp32r streams at 1.5 cyc/row on TRN2; bf16 = 1.0 and
   enables the compiler's fast-weight-load path).
 - Waves of 2 atom-pairs over 512-struct chunks. Layer-1 matmuls are
   row-tiled (K=11 per pair at partition offsets 32*rg) so two/four K=11
   matmuls run concurrently in the PE array.
 - Layer 3 (e = W3.T h2) runs as a chunk-end burst of column-tiled matmuls:
   4 concurrent M=32 matmuls (one per 32-atom column group), 16 accumulation
   steps, writing the [128 atoms, 512 structs] block in one PSUM bank. The
   bank borrows a z1 pool slot for ~4us at each chunk boundary, keeping total
   PSUM usage at 8 banks (z1 ring 4 + z2 ring 4).
 - tanh1 on the ACT engine (exact, table-based). tanh2 mostly on the Vector
   engine via a custom fused DVE op: clip(x*((a*u + b)*u + c), -s, s) with
   u = x^2 -- an 8-stage minimax fit good to ~6e-3 RMS over the z2
   distribution; every few waves tanh2 goes to ACT instead to balance load.
 - A ~4.5us matmul warm-up burst anchored on the first g-chunk DMA flips the
   PE HAM clock gate to 8/8 (2.4 GHz) right before the pipeline starts and
   the steady-state stream keeps it warm.
"""

from contextlib import ExitStack

import numpy as np

S, A, D, H = 4096, 1024, 5, 64
NCORES = 8
ACORE = A // NCORES  # 128 atoms per core
NPAIR = ACORE // 2  # 64 atom pairs per core
NS = 512  # struct chunk = one PSUM bank of fp32
NCHUNK = S // NS  # 8
KG = D * 2 + 1  # 11 rows per pair: two atoms' descriptors + ones row (b1 fold)
NWAVE = 32  # waves per chunk, 2 pairs each
NT = 16  # pairs per l3 column group

# Poly-tanh for the custom DVE op, reparameterized so the linear coefficient
# is the hardware One constant: with z2' = LAM*z2 (W2 pre-scaled host-side),
#   op(x) = clip(x*((x^2*PU - 1)*x^2 + 1), -SU, SU)  and  tanh(z2) ~= KAP*op(z2')
# (minimax quintic weighted by z2 ~ N(0, 6.8), max err ~2.6e-2, rms ~6e-3).
PU = 0.49387755656085486
SU = 0.48298912598652705
LAM = 0.4596675281296463
KAP = 2.066172035977965

ACT_TANH2_MOD = 10  # waves with w % MOD == MOD-1 run tanh2 on ACT (exact)


def _wave_of_pair(p):
    """Wave index (within a chunk) that processes pair p."""
    rg, t = divmod(p, NT)
    return t + NT * (rg // 2)


def _pair_on_act(p):
    return _wave_of_pair(p) % ACT_TANH2_MOD == ACT_TANH2_MOD - 1

_compiled = {}
_dve_registered = []


def _register_tanh_op():
    """Register the fused poly-tanh custom DVE op (runtime extension of the
    documented dve_ops registry; sha self-pinned)."""
    if _dve_registered:
        return _dve_registered[0]
    import concourse.dve_ops as dve_ops
    from concourse.dve_ops import DveOp
    from concourse.dve_spec import Spec, Src0, C0, C1, C2, One, lower, maxx, minn, sq
    from concourse.dve_uop import DveOpSpec

    name = "TANH_UNIT_ANT"
    u = sq(Src0)
    spec = Spec(
        body=minn(maxx(Src0 * ((u * C0 - One) * u + One), C1), C2),
        reference=lambda in0, s0, s1, imm2: np.minimum(
            np.maximum(
                in0.astype(np.float32)
                * (
                    (np.square(in0.astype(np.float32)) * s0 - 1.0)
                    * np.square(in0.astype(np.float32))
                    + 1.0
                ),
                s1,
            ),
            imm2,
        ),
    )
    if name not in dve_ops._SUB_OPCODE_FOR_NAME:
        row = max(dve_ops._SUB_OPCODE_FOR_NAME.values()) + 1
        assert row < 0x20
        dve_ops._SUB_OPCODE_FOR_NAME[name] = row
    shas = {}
    for ver in ("v3", "v4"):
        s = DveOpSpec(
            name=name,
            opcode=dve_ops._SUB_OPCODE_FOR_NAME[name],
            uops=lower(spec, ver=ver),
            rd1_en=False,
        )
        shas[ver] = s.sha(ver)
    op = DveOp(name, spec, subdim=False, uops_sha=shas)
    if not any(o.name == name for o in dve_ops.OPS):
        dve_ops.OPS.append(op)
    dve_ops.CUSTOM_DVE_SPECS[name] = spec
    _dve_registered.append(op)
    return op


def _build(with_b2):
    import concourse.tile as tile
    import concourse.mybir as mybir
    from concourse import bacc

    tanh_op = _register_tanh_op()

    dt = mybir.dt
    mdt = dt.bfloat16
    Tanh = mybir.ActivationFunctionType.Tanh

    nc = bacc.Bacc(
        "TRN2", target_bir_lowering=False, debug=False, num_devices=NCORES
    )
    # gt: [chunk, rowgroup, 11, 16 wave-slots, NS] (see _prep_core)
    gt = nc.declare_dram_parameter("gt", [NCHUNK, 4, KG, NT, NS], mdt, isOutput=False)
    w1 = nc.declare_dram_parameter("w1", [128, NT * 128], mdt, isOutput=False)
    w2 = nc.declare_dram_parameter("w2", [128, NPAIR * 128], mdt, isOutput=False)
    w3 = nc.declare_dram_parameter("w3", [128, NPAIR * 32], mdt, isOutput=False)
    if with_b2:
        b2d = nc.declare_dram_parameter("b2d", [128, NPAIR], dt.float32, isOutput=False)
    eo = nc.declare_dram_parameter("eo", [128, S], dt.float32, isOutput=True)

    with tile.TileContext(nc) as tc, ExitStack() as ctx:
        wp = ctx.enter_context(tc.tile_pool(name="wp", bufs=1))
        gp = ctx.enter_context(tc.tile_pool(name="gp", bufs=2))
        h1p = ctx.enter_context(tc.tile_pool(name="h1p", bufs=3))
        h2p = ctx.enter_context(tc.tile_pool(name="h2p", bufs=36))
        eop = ctx.enter_context(tc.tile_pool(name="eop", bufs=2))
        z1p = ctx.enter_context(tc.tile_pool(name="z1p", bufs=2, space="PSUM"))
        z2p = ctx.enter_context(tc.tile_pool(name="z2p", bufs=2, space="PSUM"))

        w1t = wp.tile([128, NT * 128], mdt)
        nc.sync.dma_start(w1t[:], w1[:])
        w2t = wp.tile([128, NPAIR * 128], mdt)
        nc.gpsimd.dma_start(w2t[:], w2[:])
        w3t = wp.tile([128, NPAIR * 32], mdt)
        nc.gpsimd.dma_start(w3t[:], w3[:])
        if with_b2:
            b2t = wp.tile([128, NPAIR], dt.float32)
            nc.sync.dma_start(b2t[:], b2d[:])

        gstage = {}

        def ensure_chunk(c):
            if c in gstage or c >= NCHUNK:
                return
            gs = gp.tile([128, NT * NS], mdt, name=f"gs{c}", tag="gs")
            for rg in range(4):
                nc.sync.dma_start(gs[32 * rg : 32 * rg + KG, :], gt[c, rg])
            gstage[c] = gs

        ensure_chunk(0)

        # Warm-up: ~10 back-to-back N=512 matmuls anchored on the chunk-0 g
        # DMA -- ~4.5us of dense PE work flips the HAM clock gate to 8/8
        # right before the first real wave; steady-state gaps stay under the
        # ~3.4us re-throttle window so the PE stays at 2.4 GHz.
        zwu = z1p.tile([128, 1024], dt.float32, name="zwu", tag="z1")
        for _ in range(10):
            nc.tensor.matmul(
                zwu[:, 0:NS],
                w1t[0:KG, 0:128],
                gstage[0][0:KG, 0:NS],
                start=True,
                stop=True,
            )

        h2_tiles = {}  # pair index -> (tile, col slice start)

        def stage_l1(c, w):
            """Layer-1 wave: 2 row-tiled K=11 matmuls (pairs at row groups
            (0,1) for w<16, (2,3) for w>=16) into one 2-bank z1 tile."""
            ensure_chunk(c + 1)
            gs = gstage[c]
            t = w % NT
            z1 = z1p.tile([128, 1024], dt.float32, name=f"z1_{c}_{w}", tag="z1")
            for s in range(2):
                rg = 2 * (w // NT) + s
                nc.tensor.matmul(
                    z1[:, s * NS : (s + 1) * NS],
                    w1t[32 * rg : 32 * rg + KG, t * 128 : (t + 1) * 128],
                    gs[32 * rg : 32 * rg + KG, t * NS : (t + 1) * NS],
                    start=True,
                    stop=True,
                    tile_position=(32 * rg, 0),
                )
            h1 = h1p.tile([128, 1024], mdt, name=f"h1_{c}_{w}", tag="h1")
            nc.scalar.activation(h1[:], z1[:], Tanh)
            return h1

        def stage_l2(c, w, h1, q):
            """Layer-2 wave: 2 matmuls + tanh2 (custom DVE poly op, or exact
            ACT tanh every ACT_TANH2_EVERY waves / when b2 is folded)."""
            t = w % NT
            z2 = z2p.tile([128, 1024], dt.float32, name=f"z2_{c}_{w}", tag="z2")
            for s in range(2):
                rg = 2 * (w // NT) + s
                p = NT * rg + t
                nc.tensor.matmul(
                    z2[:, s * NS : (s + 1) * NS],
                    w2t[:, p * 128 : (p + 1) * 128],
                    h1[:, s * NS : (s + 1) * NS],
                    start=True,
                    stop=True,
                )
            if with_b2:
                for s in range(2):
                    rg = 2 * (w // NT) + s
                    p = NT * rg + t
                    nc.vector.tensor_scalar_add(
                        z2[:, s * NS : (s + 1) * NS],
                        z2[:, s * NS : (s + 1) * NS],
                        b2t[:, p : p + 1],
                    )
            h2 = h2p.tile([128, 1024], mdt, name=f"h2_{c}_{w}", tag="h2")
            if w % ACT_TANH2_MOD == ACT_TANH2_MOD - 1:
                nc.scalar.activation(h2[:], z2[:], Tanh)
            else:
                nc.vector._custom_dve(
                    tanh_op, out=h2[:], in0=z2[:], s0=PU, s1=-SU, imm2=SU
                )
            for s in range(2):
                rg = 2 * (w // NT) + s
                h2_tiles[NT * rg + t] = (h2, s * NS)
            return h2

        def stage_l3(c):
            """Chunk-end burst: per accumulation step t, four column-tiled
            matmuls (one per 32-atom column group) run concurrently. PSUM
            accumulation groups are tracked per 2KB bank, so each column
            group owns its own bank (two borrowed z2-ring tiles); the four
            [32, 512] slices DMA straight from PSUM to DRAM (b3 is applied
            host-side)."""
            eta = z2p.tile([128, 1024], dt.float32, name=f"eta{c}", tag="z2")
            etb = z2p.tile([128, 1024], dt.float32, name=f"etb{c}", tag="z2")
            ets = [(eta, 0), (eta, NS), (etb, 0), (etb, NS)]
            for t in range(NT):
                for gcol in range(4):
                    p = NT * gcol + t
                    h2, off = h2_tiles[p]
                    ett, eoff = ets[gcol]
                    nc.tensor.matmul(
                        ett[32 * gcol : 32 * gcol + 32, eoff : eoff + NS],
                        w3t[:, p * 32 : (p + 1) * 32],
                        h2[:, off : off + NS],
                        start=(t == 0),
                        stop=(t == NT - 1),
                        tile_position=(0, 32 * gcol),
                    )
            h2_tiles.clear()
            eot = eop.tile([128, NS], dt.float32, name=f"eot{c}", tag="eot")
            for gcol in range(4):
                ett, eoff = ets[gcol]
                src = ett[32 * gcol : 32 * gcol + 32, eoff : eoff + NS]
                dst = eot[32 * gcol : 32 * gcol + 32, :]
                if gcol % 2 == 0:
                    nc.scalar.copy(dst, src)
                else:
                    nc.vector.tensor_scalar_add(dst, src, 0.0)
            nc.sync.dma_start(eo[:, c * NS : (c + 1) * NS], eot[:])

        # Software pipeline: wave q's layer-1 is issued before wave q-1's
        # layer-2 so every engine always has independent work queued.
        pending = None
        for q in range(NCHUNK * NWAVE):
            c, w = divmod(q, NWAVE)
            h1 = stage_l1(c, w)
            if pending is not None:
                pc, pw, ph1 = pending
                stage_l2(pc, pw, ph1, q - 1)
                if pw == NWAVE - 1:
                    stage_l3(pc)
            pending = (c, w, h1)
        pc, pw, ph1 = pending
        stage_l2(pc, pw, ph1, NCHUNK * NWAVE - 1)
        stage_l3(pc)
    nc.compile()
    return nc


def _prep_core(c, g, W1, b1, W2, b2, W3, b3, with_b2):
    import ml_dtypes

    at = slice(c * ACORE, (c + 1) * ACORE)
    f32 = np.float32
    bf16 = ml_dtypes.bfloat16

    # Pair P = atoms (2P, 2P+1); l3 column group gcol = P // 16, slot t = P % 16.
    # Wave w covers pairs (16*(2*(w//16)) + w%16, 16*(2*(w//16)+1) + w%16).
    gc = g[:, at, :]  # [S, 128, 5]
    # gt[c, rg, r, t, n]: descriptor row r of pair P = 16*rg + t at struct 512c+n.
    gT = np.ascontiguousarray(gc.transpose(1, 2, 0))  # [128, 5, S]
    gT = gT.reshape(NPAIR, 2, D, S)  # [pair, parity, d, S]
    gt = np.empty((NCHUNK, 4, KG, NT, NS), dtype=f32)
    # rows 0-4: even atom's descriptors; 5-9: odd atom's; 10: ones
    src = gT.reshape(4, NT, 2, D, NCHUNK, NS)  # [rg, t, parity, d, c, n]
    gt[:, :, 0:D] = src[:, :, 0].transpose(3, 0, 2, 1, 4)
    gt[:, :, D : 2 * D] = src[:, :, 1].transpose(3, 0, 2, 1, 4)
    gt[:, :, 2 * D] = 1.0

    W1c, b1c = W1[at], b1[at]  # [128, 5, 64], [128, 64]
    # w1t rows 32*rg + r, block t: pair P = 16*rg + t
    w1a = np.zeros((4, 32, NT, 128), dtype=f32)
    W1p = W1c.reshape(NPAIR, 2, D, H).reshape(4, NT, 2, D, H)
    b1p = b1c.reshape(NPAIR, 2, H).reshape(4, NT, 2, H)
    w1a[:, 0:D, :, 0:H] = W1p[:, :, 0].transpose(0, 2, 1, 3)
    w1a[:, D : 2 * D, :, H:] = W1p[:, :, 1].transpose(0, 2, 1, 3)
    w1a[:, 2 * D, :, 0:H] = b1p[:, :, 0]
    w1a[:, 2 * D, :, H:] = b1p[:, :, 1]
    w1d = w1a.reshape(128, NT * 128)

    # Pairs whose tanh2 runs on the DVE poly op get z2 pre-scaled by LAM
    # (via W2/b2) and the unit-poly output post-scaled by KAP (via W3).
    dve_scale = np.array(
        [1.0 if _pair_on_act(p) else LAM for p in range(NPAIR)], dtype=f32
    )

    W2c = W2[at]  # [128, 64, 64]
    w2a = np.zeros((NPAIR, 128, 128), dtype=f32)
    w2a[:, :H, :H] = W2c[0::2] * dve_scale[:, None, None]
    w2a[:, H:, H:] = W2c[1::2] * dve_scale[:, None, None]
    w2d = np.ascontiguousarray(w2a.transpose(1, 0, 2)).reshape(128, NPAIR * 128)

    W3c = W3[at][..., 0]  # [128, 64]
    w3a = np.zeros((NPAIR, 128, 32), dtype=f32)
    for p in range(NPAIR):
        t = p % NT
        k = 1.0 if _pair_on_act(p) else KAP
        w3a[p, :H, 2 * t] = W3c[2 * p] * k
        w3a[p, H:, 2 * t + 1] = W3c[2 * p + 1] * k
    w3d = np.ascontiguousarray(w3a.transpose(1, 0, 2)).reshape(128, NPAIR * 32)

    in_map = {
        "gt": np.ascontiguousarray(gt).astype(bf16),
        "w1": w1d.astype(bf16),
        "w2": w2d.astype(bf16),
        "w3": w3d.astype(bf16),
    }
    if with_b2:
        b2c = b2[at]  # [128, 64]
        in_map["b2d"] = np.ascontiguousarray(
            np.concatenate([b2c[0::2].T, b2c[1::2].T], axis=0) * dve_scale[None, :]
        ).astype(f32)
    return in_map


def kernel(g, W1, b1, W2, b2, W3, b3):
    from concourse.bass_utils import run_bass_kernel_spmd

    g = np.asarray(g, dtype=np.float32)
    W1 = np.asarray(W1, dtype=np.float32)
    b1 = np.asarray(b1, dtype=np.float32)
    W2 = np.asarray(W2, dtype=np.float32)
    b2 = np.asarray(b2, dtype=np.float32)
    W3 = np.asarray(W3, dtype=np.float32)
    b3 = np.asarray(b3, dtype=np.float32)

    with_b2 = bool(np.any(b2))
    if with_b2 not in _compiled:
        _compiled[with_b2] = _build(with_b2)
    nc = _compiled[with_b2]

    in_maps = [
        _prep_core(c, g, W1, b1, W2, b2, W3, b3, with_b2) for c in range(NCORES)
    ]
    res = run_bass_kernel_spmd(nc, in_maps, list(range(NCORES)))

    e = np.empty((S, A), dtype=np.float32)
    for c in range(NCORES):
        e[:, c * ACORE : (c + 1) * ACORE] = res.results[c]["eo"].T
    e += b3[:, 0][None, :]
    return e


# revision 32
# speedup vs baseline: 2.2846x; 1.1095x over previous
"""Trainium2 Bass kernel for grouped per-atom MLPs (AtomicNN energy eval).

Math: e[s, a] = W3[a].T tanh(W2[a].T tanh(W1[a].T g[s,a] + b1[a]) + b2[a]) + b3[a]
Shapes: g [4096, 1024, 5], per-atom MLP 5 -> 64 -> 64 -> 1.

Strategy (8 NeuronCores, SPMD, atom-sharded: core c owns atoms [128c, 128c+128)):
 - All matmuls in bf16 (fp32r streams at 1.5 cyc/row on TRN2; bf16 = 1.0 and
   enables the compiler's fast-weight-load path).
 - Waves of 2 atom-pairs over 512-struct chunks. Layer-1 matmuls are
   row-tiled (K=11 per pair at partition offsets 32*rg) so two/four K=11
   matmuls run concurrently in the PE array.
 - Layer 3 (e = W3.T h2) runs as a chunk-end burst of column-tiled matmuls:
   4 concurrent M=32 matmuls (one per 32-atom column group), 16 accumulation
   steps, writing the [128 atoms, 512 structs] block in one PSUM bank. The
   bank borrows a z1 pool slot for ~4us at each chunk boundary, keeping total
   PSUM usage at 8 banks (z1 ring 4 + z2 ring 4).
 - tanh1 on the ACT engine (exact, table-based). tanh2 mostly on the Vector
   engine via a custom fused DVE op: clip(x*((a*u + b)*u + c), -s, s) with
   u = x^2 -- an 8-stage minimax fit good to ~6e-3 RMS over the z2
   distribution; every few waves tanh2 goes to ACT instead to balance load.
 - A ~4.5us matmul warm-up burst anchored on the first g-chunk DMA flips the
   PE HAM clock gate to 8/8 (2.4 GHz) right before the pipeline starts and
   the steady-state stream keeps it warm.
"""

from contextlib import ExitStack

import numpy as np

S, A, D, H = 4096, 1024, 5, 64
NCORES = 8
ACORE = A // NCORES  # 128 atoms per core
NPAIR = ACORE // 2  # 64 atom pairs per core
NS = 512  # struct chunk = one PSUM bank of fp32
NCHUNK = S // NS  # 8
KG = D * 2 + 1  # 11 rows per pair: two atoms' descriptors + ones row (b1 fold)
NWAVE = 32  # waves per chunk, 2 pairs each
NT = 16  # pairs per l3 column group

# Poly-tanh for the custom DVE op, reparameterized so the linear coefficient
# is the hardware One constant: with z2' = LAM*z2 (W2 pre-scaled host-side),
#   op(x) = clip(x*((x^2*PU - 1)*x^2 + 1), -SU, SU)  and  tanh(z2) ~= KAP*op(z2')
# (minimax quintic weighted by z2 ~ N(0, 6.8), max err ~2.6e-2, rms ~6e-3).
PU = 0.49387755656085486
SU = 0.48298912598652705
LAM = 0.4596675281296463
KAP = 2.066172035977965

ACT_TANH2_MOD = 10  # waves with w % MOD == MOD-1 run tanh2 on ACT (exact)


def _wave_on_act(w):
    """tanh2 of wave w runs on ACT: every MOD-th wave (load balance) plus the
    last two waves of each chunk, so the layer-3 burst's PSUM-slot WAR (on
    the z2 ring) clears without waiting out the Vector engine's backlog."""
    return w % ACT_TANH2_MOD == ACT_TANH2_MOD - 1 or w >= NWAVE - 2


def _pair_on_act(p):
    rg, t = divmod(p, NT)
    return _wave_on_act(t + NT * (rg // 2))

_compiled = {}
_dve_registered = []


def _register_tanh_op():
    """Register the fused poly-tanh custom DVE op (runtime extension of the
    documented dve_ops registry; sha self-pinned)."""
    if _dve_registered:
        return _dve_registered[0]
    import concourse.dve_ops as dve_ops
    from concourse.dve_ops import DveOp
    from concourse.dve_spec import Spec, Src0, C0, C1, C2, One, lower, maxx, minn, sq
    from concourse.dve_uop import DveOpSpec

    name = "TANH_UNIT_ANT"
    u = sq(Src0)
    spec = Spec(
        body=minn(maxx(Src0 * ((u * C0 - One) * u + One), C1), C2),
        reference=lambda in0, in1, s0, s1, imm2: np.minimum(
            np.maximum(
                in0.astype(np.float32)
                * (
                    (np.square(in0.astype(np.float32)) * s0 - 1.0)
                    * np.square(in0.astype(np.float32))
                    + 1.0
                ),
                s1,
            ),
            imm2,
        ),
    )
    if name not in dve_ops._SUB_OPCODE_FOR_NAME:
        row = max(dve_ops._SUB_OPCODE_FOR_NAME.values()) + 1
        assert row < 0x20
        dve_ops._SUB_OPCODE_FOR_NAME[name] = row
    shas = {}
    for ver in ("v3", "v4"):
        s = DveOpSpec(
            name=name,
            opcode=dve_ops._SUB_OPCODE_FOR_NAME[name],
            uops=lower(spec, ver=ver),
            rd1_en=False,
        )
        shas[ver] = s.sha(ver)
    op = DveOp(name, spec, subdim=False, uops_sha=shas)
    if not any(o.name == name for o in dve_ops.OPS):
        dve_ops.OPS.append(op)
    dve_ops.CUSTOM_DVE_SPECS[name] = spec
    _dve_registered.append(op)
    return op


def _build(with_b2):
    import concourse.tile as tile
    import concourse.mybir as mybir
    from concourse import bacc

    tanh_op = _register_tanh_op()

    dt = mybir.dt
    mdt = dt.bfloat16
    Tanh = mybir.ActivationFunctionType.Tanh

    nc = bacc.Bacc(
        "TRN2", target_bir_lowering=False, debug=False, num_devices=NCORES
    )
    # gt: [chunk, rowgroup, 11, 16 wave-slots, NS] (see _prep_core)
    gt = nc.declare_dram_parameter("gt", [NCHUNK, 4, KG, NT, NS], mdt, isOutput=False)
    w1 = nc.declare_dram_parameter("w1", [128, NT * 128], mdt, isOutput=False)
    w2 = nc.declare_dram_parameter("w2", [128, NPAIR * 128], mdt, isOutput=False)
    w3 = nc.declare_dram_parameter("w3", [128, NPAIR * 32], mdt, isOutput=False)
    if with_b2:
        b2d = nc.declare_dram_parameter("b2d", [128, NPAIR], dt.float32, isOutput=False)
    eo = nc.declare_dram_parameter("eo", [128, S], dt.float32, isOutput=True)

    with tile.TileContext(nc) as tc, ExitStack() as ctx:
        wp = ctx.enter_context(tc.tile_pool(name="wp", bufs=1))
        gp = ctx.enter_context(tc.tile_pool(name="gp", bufs=2))
        h1p = ctx.enter_context(tc.tile_pool(name="h1p", bufs=4))
        h2p = ctx.enter_context(tc.tile_pool(name="h2p", bufs=36))
        eop = ctx.enter_context(tc.tile_pool(name="eop", bufs=2))
        z1p = ctx.enter_context(tc.tile_pool(name="z1p", bufs=2, space="PSUM"))
        z2p = ctx.enter_context(tc.tile_pool(name="z2p", bufs=2, space="PSUM"))

        w1t = wp.tile([128, NT * 128], mdt)
        nc.sync.dma_start(w1t[:], w1[:])
        w2t = wp.tile([128, NPAIR * 128], mdt)
        nc.gpsimd.dma_start(w2t[:], w2[:])
        w3t = wp.tile([128, NPAIR * 32], mdt)
        nc.gpsimd.dma_start(w3t[:], w3[:])
        if with_b2:
            b2t = wp.tile([128, NPAIR], dt.float32)
            nc.sync.dma_start(b2t[:], b2d[:])

        gstage = {}

        def ensure_chunk(c):
            if c in gstage or c >= NCHUNK:
                return
            gs = gp.tile([128, NT * NS], mdt, name=f"gs{c}", tag="gs")
            for rg in range(4):
                nc.sync.dma_start(gs[32 * rg : 32 * rg + KG, :], gt[c, rg])
            gstage[c] = gs

        ensure_chunk(0)

        # Warm-up: ~10 back-to-back N=512 matmuls anchored on the chunk-0 g
        # DMA -- ~4.5us of dense PE work flips the HAM clock gate to 8/8
        # right before the first real wave; steady-state gaps stay under the
        # ~3.4us re-throttle window so the PE stays at 2.4 GHz.
        zwu = z1p.tile([128, 1024], dt.float32, name="zwu", tag="z1")
        for _ in range(14):
            nc.tensor.matmul(
                zwu[:, 0:NS],
                w1t[0:KG, 0:128],
                gstage[0][0:KG, 0:NS],
                start=True,
                stop=True,
            )

        h2_tiles = {}  # pair index -> (tile, col slice start)

        def stage_l1(c, w):
            """Layer-1 wave: 2 row-tiled K=11 matmuls (pairs at row groups
            (0,1) for w<16, (2,3) for w>=16) into one 2-bank z1 tile."""
            ensure_chunk(c + 1)
            gs = gstage[c]
            t = w % NT
            z1 = z1p.tile([128, 1024], dt.float32, name=f"z1_{c}_{w}", tag="z1")
            for s in range(2):
                rg = 2 * (w // NT) + s
                nc.tensor.matmul(
                    z1[:, s * NS : (s + 1) * NS],
                    w1t[32 * rg : 32 * rg + KG, t * 128 : (t + 1) * 128],
                    gs[32 * rg : 32 * rg + KG, t * NS : (t + 1) * NS],
                    start=True,
                    stop=True,
                    tile_position=(32 * rg, 0),
                )
            h1 = h1p.tile([128, 1024], mdt, name=f"h1_{c}_{w}", tag="h1")
            nc.scalar.activation(h1[:], z1[:], Tanh)
            return h1

        def stage_l2(c, w, h1, q):
            """Layer-2 wave: 2 matmuls + tanh2 (custom DVE poly op, or exact
            ACT tanh every ACT_TANH2_EVERY waves / when b2 is folded)."""
            t = w % NT
            z2 = z2p.tile([128, 1024], dt.float32, name=f"z2_{c}_{w}", tag="z2")
            for s in range(2):
                rg = 2 * (w // NT) + s
                p = NT * rg + t
                nc.tensor.matmul(
                    z2[:, s * NS : (s + 1) * NS],
                    w2t[:, p * 128 : (p + 1) * 128],
                    h1[:, s * NS : (s + 1) * NS],
                    start=True,
                    stop=True,
                )
            if with_b2:
                for s in range(2):
                    rg = 2 * (w // NT) + s
                    p = NT * rg + t
                    nc.vector.tensor_scalar_add(
                        z2[:, s * NS : (s + 1) * NS],
                        z2[:, s * NS : (s + 1) * NS],
                        b2t[:, p : p + 1],
                    )
            h2 = h2p.tile([128, 1024], mdt, name=f"h2_{c}_{w}", tag="h2")
            if _wave_on_act(w):
                nc.scalar.activation(h2[:], z2[:], Tanh)
            else:
                nc.vector._custom_dve(
                    tanh_op, out=h2[:], in0=z2[:], s0=PU, s1=-SU, imm2=SU
                )
            for s in range(2):
                rg = 2 * (w // NT) + s
                h2_tiles[NT * rg + t] = (h2, s * NS)
            return h2

        def stage_l3(c):
            """Chunk-end burst: per accumulation step t, four column-tiled
            matmuls (one per 32-atom column group) run concurrently. PSUM
            accumulation groups are tracked per 2KB bank, so each column
            group owns its own bank (two borrowed z2-ring tiles); the four
            [32, 512] slices DMA straight from PSUM to DRAM (b3 is applied
            host-side)."""
            eta = z2p.tile([128, 1024], dt.float32, name=f"eta{c}", tag="z2")
            etb = z2p.tile([128, 1024], dt.float32, name=f"etb{c}", tag="z2")
            ets = [(eta, 0), (eta, NS), (etb, 0), (etb, NS)]
            for t in range(NT):
                for gcol in range(4):
                    p = NT * gcol + t
                    h2, off = h2_tiles[p]
                    ett, eoff = ets[gcol]
                    nc.tensor.matmul(
                        ett[32 * gcol : 32 * gcol + 32, eoff : eoff + NS],
                        w3t[:, p * 32 : (p + 1) * 32],
                        h2[:, off : off + NS],
                        start=(t == 0),
                        stop=(t == NT - 1),
                        tile_position=(0, 32 * gcol),
                    )
            h2_tiles.clear()
            eot = eop.tile([128, NS], dt.float32, name=f"eot{c}", tag="eot")
            for gcol in range(4):
                ett, eoff = ets[gcol]
                src = ett[32 * gcol : 32 * gcol + 32, eoff : eoff + NS]
                dst = eot[32 * gcol : 32 * gcol + 32, :]
                if gcol % 2 == 0:
                    nc.scalar.copy(dst, src)
                else:
                    nc.vector.tensor_scalar_add(dst, src, 0.0)
            nc.sync.dma_start(eo[:, c * NS : (c + 1) * NS], eot[:])

        # Software pipeline, layer-1 two waves ahead of layer-2 (the z1 ring
        # holds exactly two in-flight waves), so the PE always has queued
        # work across chunk boundaries.
        from collections import deque

        pending = deque()
        for q in range(NCHUNK * NWAVE):
            c, w = divmod(q, NWAVE)
            h1 = stage_l1(c, w)
            pending.append((c, w, h1))
            if len(pending) > 2:
                pc, pw, ph1 = pending.popleft()
                stage_l2(pc, pw, ph1, 0)
                if pw == NWAVE - 1:
                    stage_l3(pc)
        while pending:
            pc, pw, ph1 = pending.popleft()
            stage_l2(pc, pw, ph1, 0)
            if pw == NWAVE - 1:
                stage_l3(pc)
    nc.compile()
    return nc


def _prep_core(c, g, W1, b1, W2, b2, W3, b3, with_b2):
    import ml_dtypes

    at = slice(c * ACORE, (c + 1) * ACORE)
    f32 = np.float32
    bf16 = ml_dtypes.bfloat16

    # Pair P = atoms (2P, 2P+1); l3 column group gcol = P // 16, slot t = P % 16.
    # Wave w covers pairs (16*(2*(w//16)) + w%16, 16*(2*(w//16)+1) + w%16).
    gc = g[:, at, :]  # [S, 128, 5]
    # gt[c, rg, r, t, n]: descriptor row r of pair P = 16*rg + t at struct 512c+n.
    gT = np.ascontiguousarray(gc.transpose(1, 2, 0))  # [128, 5, S]
    gT = gT.reshape(NPAIR, 2, D, S)  # [pair, parity, d, S]
    gt = np.empty((NCHUNK, 4, KG, NT, NS), dtype=f32)
    # rows 0-4: even atom's descriptors; 5-9: odd atom's; 10: ones
    src = gT.reshape(4, NT, 2, D, NCHUNK, NS)  # [rg, t, parity, d, c, n]
    gt[:, :, 0:D] = src[:, :, 0].transpose(3, 0, 2, 1, 4)
    gt[:, :, D : 2 * D] = src[:, :, 1].transpose(3, 0, 2, 1, 4)
    gt[:, :, 2 * D] = 1.0

    W1c, b1c = W1[at], b1[at]  # [128, 5, 64], [128, 64]
    # w1t rows 32*rg + r, block t: pair P = 16*rg + t
    w1a = np.zeros((4, 32, NT, 128), dtype=f32)
    W1p = W1c.reshape(NPAIR, 2, D, H).reshape(4, NT, 2, D, H)
    b1p = b1c.reshape(NPAIR, 2, H).reshape(4, NT, 2, H)
    w1a[:, 0:D, :, 0:H] = W1p[:, :, 0].transpose(0, 2, 1, 3)
    w1a[:, D : 2 * D, :, H:] = W1p[:, :, 1].transpose(0, 2, 1, 3)
    w1a[:, 2 * D, :, 0:H] = b1p[:, :, 0]
    w1a[:, 2 * D, :, H:] = b1p[:, :, 1]
    w1d = w1a.reshape(128, NT * 128)

    # Pairs whose tanh2 runs on the DVE poly op get z2 pre-scaled by LAM
    # (via W2/b2) and the unit-poly output post-scaled by KAP (via W3).
    dve_scale = np.array(
        [1.0 if _pair_on_act(p) else LAM for p in range(NPAIR)], dtype=f32
    )

    W2c = W2[at]  # [128, 64, 64]
    w2a = np.zeros((NPAIR, 128, 128), dtype=f32)
    w2a[:, :H, :H] = W2c[0::2] * dve_scale[:, None, None]
    w2a[:, H:, H:] = W2c[1::2] * dve_scale[:, None, None]
    w2d = np.ascontiguousarray(w2a.transpose(1, 0, 2)).reshape(128, NPAIR * 128)

    W3c = W3[at][..., 0]  # [128, 64]
    w3a = np.zeros((NPAIR, 128, 32), dtype=f32)
    for p in range(NPAIR):
        t = p % NT
        k = 1.0 if _pair_on_act(p) else KAP
        w3a[p, :H, 2 * t] = W3c[2 * p] * k
        w3a[p, H:, 2 * t + 1] = W3c[2 * p + 1] * k
    w3d = np.ascontiguousarray(w3a.transpose(1, 0, 2)).reshape(128, NPAIR * 32)

    in_map = {
        "gt": np.ascontiguousarray(gt).astype(bf16),
        "w1": w1d.astype(bf16),
        "w2": w2d.astype(bf16),
        "w3": w3d.astype(bf16),
    }
    if with_b2:
        b2c = b2[at]  # [128, 64]
        in_map["b2d"] = np.ascontiguousarray(
            np.concatenate([b2c[0::2].T, b2c[1::2].T], axis=0) * dve_scale[None, :]
        ).astype(f32)
    return in_map


def kernel(g, W1, b1, W2, b2, W3, b3):
    from concourse.bass_utils import run_bass_kernel_spmd

    g = np.asarray(g, dtype=np.float32)
    W1 = np.asarray(W1, dtype=np.float32)
    b1 = np.asarray(b1, dtype=np.float32)
    W2 = np.asarray(W2, dtype=np.float32)
    b2 = np.asarray(b2, dtype=np.float32)
    W3 = np.asarray(W3, dtype=np.float32)
    b3 = np.asarray(b3, dtype=np.float32)

    with_b2 = bool(np.any(b2))
    if with_b2 not in _compiled:
        _compiled[with_b2] = _build(with_b2)
    nc = _compiled[with_b2]

    in_maps = [
        _prep_core(c, g, W1, b1, W2, b2, W3, b3, with_b2) for c in range(NCORES)
    ]
    res = run_bass_kernel_spmd(nc, in_maps, list(range(NCORES)))

    e = np.empty((S, A), dtype=np.float32)
    for c in range(NCORES):
        e[:, c * ACORE : (c + 1) * ACORE] = res.results[c]["eo"].T
    e += b3[:, 0][None, :]
    return e
